# revision 45
# baseline (speedup 1.0000x reference)
"""GAT forward on 8 Trainium2 NeuronCores — one attention head per core.

Math (per head, all [4096] nodes):
    h    = x @ W                    [N, 128]
    ci   = h @ w_i  (per-node)      [N]
    cj   = h @ w_j  (per-node)      [N]
    z    = leaky_relu(ci[i] + cj[j] + m[j,i])   (m = 0 / -60 additive mask)
    e    = exp(z)  (Schraudolph bit-trick on DVE, bf16)
    yT[f,i] = sum_j h[j,f] * e[j,i]            (PE matmul, e moving)
    rs[i]   = sum_j e[j,i]                     (PE matmul vs ones4 stationary)
    out[f,i] = yT[f,i]/rs[i] + residT[f,i];  host transposes to [i,f].

Engine assignment per attention tile [128j x 2048i] (64 tiles):
  - DVE:  w = mask + ciB          (tensor_tensor bf16, 2x mode)
          e = int16(z*A + B)      (tensor_scalar bf16->int16, Schraudolph exp;
                                   bitcast to bf16 feeds the PE directly)
  - ACT:  z = Prelu(w + cj_bias)  (one op does the cj add AND the leaky relu)
  - PE:   4x yT matmuls + 4x rowsum matmuls (all bf16, 512-col chunks)
  - Pool: mask DMA issue + ci/recip partition broadcasts
All matmuls everywhere are bf16 (1 cyc/row); phase 1 computes hT and residT
with the same streamed xT tiles. rowsum lands in one PSUM bank via a [4,512]
layout (block-diagonal ones stationary). Host does transpose/concat/bias.
"""
import sys

sys.path.insert(0, "/opt/trn_rl_repo")
from contextlib import ExitStack

import numpy as np
import ml_dtypes

import concourse.bass as bass
import concourse.tile as tile
from concourse import bacc, mybir
from concourse.bass_utils import run_bass_kernel_spmd

dt = mybir.dt
F32, BF16, I16 = dt.float32, dt.bfloat16, dt.int16
AF = mybir.ActivationFunctionType
OP = mybir.AluOpType

N = 4096
IN_F = 512
HF = 128
HEADS = 8
SLOPE = 0.2
MASK_NEG = -60.0
HALF = 2048
NJT = N // 128  # 32 j-tiles
NMC = IN_F // 128  # 4 contraction chunks
A_EXP = 184.6650292  # 128 * log2(e)
B_EXP = 16248.58  # 127*128 - schraudolph correction (round-to-nearest)

_prog = None


def build_program():
    nc = bacc.Bacc("TRN2", target_bir_lowering=False, debug=False)
    xT_d = nc.dram_tensor("xT", [IN_F, N], BF16, kind="ExternalInput").ap()
    mask_d = nc.dram_tensor("mask", [N, N], BF16, kind="ExternalInput").ap()
    W_d = nc.dram_tensor("W", [IN_F, HF], BF16, kind="ExternalInput").ap()
    Wr_d = nc.dram_tensor("Wr", [IN_F, HF], BF16, kind="ExternalInput").ap()
    Wwi_d = nc.dram_tensor("Wwi", [IN_F, 1], BF16, kind="ExternalInput").ap()
    wi_d = nc.dram_tensor("wi", [HF, 1], BF16, kind="ExternalInput").ap()
    wj2_d = nc.dram_tensor("wj2", [HF, 2], BF16, kind="ExternalInput").ap()
    ones_d = nc.dram_tensor("ones", [128, 1], BF16, kind="ExternalInput").ap()
    eye_d = nc.dram_tensor("eye", [128, 128], BF16, kind="ExternalInput").ap()
    y_d = nc.dram_tensor("y", [HF, N], BF16, kind="ExternalOutput").ap()

    with tile.TileContext(nc) as tc, ExitStack() as ctx:
        persist = ctx.enter_context(tc.tile_pool(name="persist", bufs=1))
        h_sb = persist.tile([128, N], BF16, tag="h")  # h[j,f] per j-tile
        rT_sb = persist.tile([128, N], BF16, tag="rT")  # residT[f,i]
        ciB = persist.tile([128, N], BF16, tag="ciB")  # ci bcast along partitions
        cjT = persist.tile([128, 2 * NJT], F32, tag="cjT")  # cj at even cols
        ones_sb = persist.tile([128, 1], BF16, tag="ones")
        eye_sb = persist.tile([128, 128], BF16, tag="eye")
        nc.gpsimd.dma_start(ones_sb[:], ones_d)
        nc.gpsimd.dma_start(eye_sb[:], eye_d)

        # Phase-2 pools opened first so their SBUF is disjoint from phase-1
        # scoped buffers.
        mpool = ctx.enter_context(tc.tile_pool(name="mpool", bufs=8))
        wpool = ctx.enter_context(tc.tile_pool(name="wpool", bufs=5))
        zpool = ctx.enter_context(tc.tile_pool(name="zpool", bufs=5))
        epool = ctx.enter_context(tc.tile_pool(name="epool", bufs=5))
        fin = ctx.enter_context(tc.tile_pool(name="fin", bufs=2))

        # ---------- Phase 1: hT/residT over streamed xT; ci/cj/h ----------
        with ExitStack() as p1:
            ph1 = p1.enter_context(tc.tile_pool(name="ph1", bufs=1))
            xpool = p1.enter_context(tc.tile_pool(name="xpool", bufs=5))
            hTp = p1.enter_context(tc.tile_pool(name="hTp", bufs=2))

            # Sync queue: x tiles (PE-critical) then the mask stream; all
            # small constants ride the gpsimd queue.
            W_sb = ph1.tile([128, NMC * HF], BF16, tag="W")
            Wr_sb = ph1.tile([128, NMC * HF], BF16, tag="Wr")
            Wwi_sb = ph1.tile([128, NMC], BF16, tag="Wwi")
            xt0 = xpool.tile([128, HALF], BF16, tag="xt")
            nc.sync.dma_start(xt0[:, 0:1024], xT_d[0:128, 0:1024])
            nc.scalar.dma_start(xt0[:, 1024:HALF], xT_d[0:128, 1024:HALF])
            for mc in range(NMC):
                nc.gpsimd.dma_start(
                    W_sb[:, mc * HF : (mc + 1) * HF], W_d[mc * 128 : (mc + 1) * 128, :]
                )
                nc.gpsimd.dma_start(
                    Wwi_sb[:, mc : mc + 1], Wwi_d[mc * 128 : (mc + 1) * 128, :]
                )
            for mc in range(NMC):
                nc.gpsimd.dma_start(
                    Wr_sb[:, mc * HF : (mc + 1) * HF],
                    Wr_d[mc * 128 : (mc + 1) * 128, :],
                )
            wi_sb = ph1.tile([128, 1], BF16, tag="wi")
            nc.gpsimd.dma_start(wi_sb[:], wi_d)
            wj2_sb = ph1.tile([128, 2], BF16, tag="wj2")
            nc.gpsimd.dma_start(wj2_sb[:], wj2_d)

            for hf in range(2):
                o = hf * HALF
                hT_sb = hTp.tile([128, HALF], BF16, tag="hT")
                xts = []
                # Loop 1: hT. For half 0 only, ci rides in-stream (via the
                # host-precomputed W@w_i) so ciB is ready early and phase-2
                # elementwise starts while phase 1 still owns the PE. Half
                # 1's ci deadline is loose, so it skips the extra 4 PSUM
                # banks here (faster pool handover from the previous loop).
                with ExitStack() as ps1:
                    psA = ps1.enter_context(
                        tc.tile_pool(name=f"psA{hf}", bufs=1, space="PSUM")
                    )
                    ps_hT = psA.tile([128, HALF], F32, tag="ps_hT")
                    ps_ci = (
                        psA.tile([1, HALF], F32, tag="ps_ci", name="ps_ci")
                        if hf == 0
                        else None
                    )
                    for mc in range(NMC):
                        if hf == 0 and mc == 0:
                            xt = xt0
                        else:
                            xt = xpool.tile([128, HALF], BF16, tag="xt")
                            nc.sync.dma_start(
                                xt[:], xT_d[mc * 128 : (mc + 1) * 128, o : o + HALF]
                            )
                        xts.append(xt)
                        for ck in range(HALF // 512):
                            nc.tensor.matmul(
                                ps_hT[:, ck * 512 : (ck + 1) * 512],
                                W_sb[:, mc * HF : (mc + 1) * HF],
                                xt[:, ck * 512 : (ck + 1) * 512],
                                start=(mc == 0),
                                stop=(mc == NMC - 1),
                            )
                        if hf == 0:
                            for ck in range(HALF // 512):
                                nc.tensor.matmul(
                                    ps_ci[0:1, ck * 512 : (ck + 1) * 512],
                                    Wwi_sb[:, mc : mc + 1],
                                    xt[:, ck * 512 : (ck + 1) * 512],
                                    start=(mc == 0),
                                    stop=(mc == NMC - 1),
                                )
                    # Evacuate split across engines so the PSUM frees fast.
                    nc.scalar.copy(hT_sb[:, 0:1024], ps_hT[:, 0:1024])
                    nc.vector.tensor_copy(hT_sb[:, 1024:HALF], ps_hT[:, 1024:HALF])
                    if hf == 0:
                        ci_row = ph1.tile([1, HALF], BF16, tag="ci_row0")
                        nc.vector.tensor_copy(ci_row[:], ps_ci[:])
                        for c in range(4):
                            nc.gpsimd.partition_broadcast(
                                ciB[:, o + c * 512 : o + (c + 1) * 512],
                                ci_row[0:1, c * 512 : (c + 1) * 512],
                            )

                # Loop 2: residT (reusing the resident x tiles), cj, h, and
                # (half 1) ci. ci comes last so its PSUM allocation lands in
                # the banks freed by the rT evacuation without stalling PE.
                with ExitStack() as ps2:
                    psB = ps2.enter_context(
                        tc.tile_pool(name=f"psB{hf}", bufs=1, space="PSUM")
                    )
                    ps_rT = psB.tile([128, HALF], F32, tag="ps_rT")
                    for mc in range(NMC):
                        for ck in range(HALF // 512):
                            nc.tensor.matmul(
                                ps_rT[:, ck * 512 : (ck + 1) * 512],
                                Wr_sb[:, mc * HF : (mc + 1) * HF],
                                xts[mc][:, ck * 512 : (ck + 1) * 512],
                                start=(mc == 0),
                                stop=(mc == NMC - 1),
                            )
                    nc.scalar.copy(rT_sb[:, o : o + 1024], ps_rT[:, 0:1024])
                    nc.vector.tensor_copy(
                        rT_sb[:, o + 1024 : o + HALF], ps_rT[:, 1024:HALF]
                    )

                    ps_cj = psB.tile([128, NJT], F32, tag="ps_cj")
                    for k in range(NJT // 2):
                        nc.tensor.matmul(
                            ps_cj[:, 2 * k : 2 * k + 2],
                            hT_sb[:, k * 128 : (k + 1) * 128],
                            wj2_sb[:],
                            start=(k == 0),
                            stop=(k == NJT // 2 - 1),
                        )
                    nc.vector.tensor_copy(cjT[:, hf * NJT : (hf + 1) * NJT], ps_cj[:])

                    ps_h = psB.tile([128, HALF], BF16, tag="ps_h")
                    for k in range(HALF // 128):
                        nc.tensor.transpose(
                            ps_h[:, k * 128 : (k + 1) * 128],
                            hT_sb[:, k * 128 : (k + 1) * 128],
                            eye_sb[:],
                        )
                    # bf16 PSUM source keeps this copy in the DVE 2x path,
                    # running parallel to the rT evacuation on ACT.
                    nc.vector.tensor_copy(h_sb[:, o : o + HALF], ps_h[:])

                if hf == 1:
                    with ExitStack() as ps3:
                        psC = ps3.enter_context(
                            tc.tile_pool(name="psC", bufs=1, space="PSUM")
                        )
                        ps_ci1 = psC.tile([1, HALF], F32, tag="ps_ci1")
                        for c in range(4):
                            nc.tensor.matmul(
                                ps_ci1[0:1, c * 512 : (c + 1) * 512],
                                wi_sb[:],
                                hT_sb[:, c * 512 : (c + 1) * 512],
                                start=True,
                                stop=True,
                            )
                        ci_row1 = ph1.tile([1, HALF], BF16, tag="ci_row1")
                        nc.vector.tensor_copy(ci_row1[:], ps_ci1[:])
                        for c in range(4):
                            nc.gpsimd.partition_broadcast(
                                ciB[:, o + c * 512 : o + (c + 1) * 512],
                                ci_row1[0:1, c * 512 : (c + 1) * 512],
                            )

        # ---------- Phase 2: attention ----------
        for half in range(2):
            i0 = half * HALF
            with ExitStack() as pmm_ctx:
                pmm = pmm_ctx.enter_context(
                    tc.tile_pool(name=f"pmm{half}", bufs=1, space="PSUM")
                )
                # yT as four chunk tiles so boundary evacuations pipeline
                # bank-by-bank instead of waiting on one big tile.
                yT_ps = [
                    pmm.tile([128, 512], F32, tag=f"yT{c}", name=f"yT_ps{c}")
                    for c in range(4)
                ]
                rs_ps = pmm.tile([1, HALF], F32, tag="rs")

                for jt in range(NJT):
                    m_t = mpool.tile([128, HALF], BF16, tag="m")
                    nc.sync.dma_start(
                        m_t[:], mask_d[jt * 128 : (jt + 1) * 128, i0 : i0 + HALF]
                    )
                    w_t = wpool.tile([128, HALF], BF16, tag="w")
                    nc.vector.tensor_tensor(
                        w_t[:], m_t[:], ciB[:, i0 : i0 + HALF], op=OP.add
                    )
                    z_t = zpool.tile([128, HALF], BF16, tag="z")
                    nc.scalar.activation(
                        z_t[:],
                        w_t[:],
                        AF.Prelu,
                        bias=cjT[:, 2 * jt : 2 * jt + 1],
                        alpha=SLOPE,
                    )
                    e_t = epool.tile([128, HALF], I16, tag="e")
                    nc.vector.tensor_scalar(
                        e_t[:], z_t[:], A_EXP, B_EXP, op0=OP.mult, op1=OP.add
                    )
                    e_bf = e_t[:].bitcast(BF16)
                    hr = h_sb[:, jt * 128 : (jt + 1) * 128]
                    # rs first so the final rowsum (tail-critical for the
                    # reciprocal) completes before the last yT matmuls.
                    for c in range(HALF // 512):
                        nc.tensor.matmul(
                            rs_ps[0:1, c * 512 : (c + 1) * 512],
                            ones_sb[:],
                            e_bf[:, c * 512 : (c + 1) * 512],
                            start=(jt == 0),
                            stop=(jt == NJT - 1),
                        )
                    for c in range(HALF // 512):
                        nc.tensor.matmul(
                            yT_ps[c][:],
                            hr,
                            e_bf[:, c * 512 : (c + 1) * 512],
                            start=(jt == 0),
                            stop=(jt == NJT - 1),
                        )

                # Finale: approx-recip the rowsums, broadcast, normalize, DMA.
                yT_sb = fin.tile([128, HALF], BF16, tag="yT_sb")
                for c in range(4):
                    nc.scalar.copy(yT_sb[:, c * 512 : (c + 1) * 512], yT_ps[c][:])
                recip_row = fin.tile([1, HALF], F32, tag="recip_row")
                nc.vector.reciprocal_approx_fast(recip_row[:], rs_ps[0:1, :])
                recipB = fin.tile([128, HALF], F32, tag="recipB")
                for c in range(4):
                    sl = slice(c * 512, (c + 1) * 512)
                    nc.gpsimd.partition_broadcast(
                        recipB[:, sl], recip_row[0:1, sl]
                    )
                ytn = fin.tile([128, HALF], BF16, tag="ytn")
                for c in range(4):
                    sl = slice(c * 512, (c + 1) * 512)
                    nc.vector.tensor_tensor(
                        ytn[:, sl], yT_sb[:, sl], recipB[:, sl], op=OP.mult
                    )
                    nc.vector.tensor_tensor(
                        ytn[:, sl],
                        ytn[:, sl],
                        rT_sb[:, i0 + c * 512 : i0 + (c + 1) * 512],
                        op=OP.add,
                    )
                    nc.scalar.dma_start(
                        y_d[:, i0 + c * 512 : i0 + (c + 1) * 512], ytn[:, sl]
                    )

    nc.compile()
    return nc


def _get_program():
    global _prog
    if _prog is None:
        _prog = build_program()
    return _prog


def _prepare_in_maps(x, graph, W, w_i, w_j, W_r):
    bf = ml_dtypes.bfloat16
    xT = np.ascontiguousarray(x.T).astype(bf)
    mask = np.where(graph > 0, np.float32(0.0), np.float32(MASK_NEG)).astype(bf)
    eye = np.eye(128, dtype=np.float32).astype(bf)
    ones = np.ones((128, 1), dtype=np.float32).astype(bf)
    in_maps = []
    for c in range(HEADS):
        wj2 = np.zeros((HF, 2), dtype=np.float32)
        wj2[:, 0] = np.asarray(w_j[c], dtype=np.float32).reshape(HF)
        in_maps.append(
            {
                "xT": xT,
                "mask": mask,
                "W": np.ascontiguousarray(W[c]).astype(bf),
                "Wr": np.ascontiguousarray(W_r[:, c * HF : (c + 1) * HF]).astype(bf),
                "Wwi": (
                    np.asarray(W[c], dtype=np.float32)
                    @ np.asarray(w_i[c], dtype=np.float32)
                ).astype(bf),
                "wi": np.asarray(w_i[c], dtype=np.float32).astype(bf),
                "wj2": wj2.astype(bf),
                "ones": ones,
                "eye": eye,
            }
        )
    return in_maps


def run(inputs, trace=False, **kwargs):
    """Run the SPMD kernel; returns (y_full, BassKernelResults)."""
    x = np.asarray(inputs["x"], dtype=np.float32)
    graph = np.asarray(inputs["graph"])
    W = np.asarray(inputs["W"], dtype=np.float32)
    w_i = np.asarray(inputs["w_i"], dtype=np.float32)
    w_j = np.asarray(inputs["w_j"], dtype=np.float32)
    W_r = np.asarray(inputs["W_r"], dtype=np.float32)
    bias = np.asarray(inputs["bias"], dtype=np.float32)

    nc = _get_program()
    in_maps = _prepare_in_maps(x, graph, W, w_i, w_j, W_r)
    br = run_bass_kernel_spmd(
        nc, in_maps, core_ids=list(range(HEADS)), trace=trace, **kwargs
    )
    y = np.concatenate(
        [br.results[c]["y"].astype(np.float32).T for c in range(HEADS)], axis=1
    )
    y = y + bias[None, :]
    return y.astype(np.float32), br


def kernel(**inputs):
    y, _ = run(inputs)
    return y


# revision 46
# speedup vs baseline: 1.0519x; 1.0519x over previous
"""GAT forward on 8 Trainium2 NeuronCores — one attention head per core.

Math (per head, all [4096] nodes):
    h    = x @ W                    [N, 128]
    ci   = h @ w_i  (per-node)      [N]
    cj   = h @ w_j  (per-node)      [N]
    z    = leaky_relu(ci[i] + cj[j] + m[j,i])   (m = 0 / -60 additive mask)
    e    = exp(z)  (Schraudolph bit-trick on DVE, bf16)
    yT[f,i] = sum_j h[j,f] * e[j,i]            (PE matmul, e moving)
    rs[i]   = sum_j e[j,i]                     (PE matmul vs block-one-hot
                                                ones4 stationary -> [4,512],
                                                one PSUM bank)
    out[f,i] = yT[f,i]/rs[i] + residT[f,i];  host transposes to [i,f].

Engine assignment per attention tile [128j x 2048i] (64 tiles):
  - DVE:  w = mask + ciB          (tensor_tensor bf16, 2x mode)
          e = int16(z*A + B)      (tensor_scalar bf16->int16, Schraudolph exp;
                                   bitcast to bf16 feeds the PE directly)
  - ACT:  z = Prelu(w + cj_bias)  (one op does the cj add AND the leaky relu)
  - PE:   4x yT matmuls + 4x rowsum matmuls (bf16, 512-col chunks), plus the
          residT GEMM chunks slotted into spare cycles (PSUM has 3 free banks
          thanks to the 1-bank rowsum)
  - Pool: ciB partition broadcasts only
The finale broadcasts 1/rs via PE outer-products (row-one-hot bc4 stationary)
so the tail never touches the slow Pool broadcast. Host does transpose/
concat/bias. All matmuls bf16 (1 cyc/row at full clock).
"""
import sys

sys.path.insert(0, "/opt/trn_rl_repo")
from contextlib import ExitStack

import numpy as np
import ml_dtypes

import concourse.bass as bass
import concourse.tile as tile
from concourse import bacc, mybir
from concourse.bass_utils import run_bass_kernel_spmd

dt = mybir.dt
F32, BF16, I16 = dt.float32, dt.bfloat16, dt.int16
AF = mybir.ActivationFunctionType
OP = mybir.AluOpType

N = 4096
IN_F = 512
HF = 128
HEADS = 8
SLOPE = 0.2
MASK_NEG = -60.0
HALF = 2048
NJT = N // 128  # 32 j-tiles
NMC = IN_F // 128  # 4 contraction chunks
A_EXP = 184.6650292  # 128 * log2(e)
B_EXP = 16248.58  # 127*128 - schraudolph correction (round-to-nearest)

_prog = None


def build_program():
    nc = bacc.Bacc("TRN2", target_bir_lowering=False, debug=False)
    xT_d = nc.dram_tensor("xT", [IN_F, N], BF16, kind="ExternalInput").ap()
    mask_d = nc.dram_tensor("mask", [N, N], BF16, kind="ExternalInput").ap()
    W_d = nc.dram_tensor("W", [IN_F, HF], BF16, kind="ExternalInput").ap()
    Wr_d = nc.dram_tensor("Wr", [IN_F, HF], BF16, kind="ExternalInput").ap()
    Wwi_d = nc.dram_tensor("Wwi", [IN_F, 1], BF16, kind="ExternalInput").ap()
    wi_d = nc.dram_tensor("wi", [HF, 1], BF16, kind="ExternalInput").ap()
    wj2_d = nc.dram_tensor("wj2", [HF, 2], BF16, kind="ExternalInput").ap()
    ones4_d = nc.dram_tensor("ones4", [128, 16], BF16, kind="ExternalInput").ap()
    bc4_d = nc.dram_tensor("bc4", [4, 512], BF16, kind="ExternalInput").ap()
    eye_d = nc.dram_tensor("eye", [128, 128], BF16, kind="ExternalInput").ap()
    y_d = nc.dram_tensor("y", [HF, N], BF16, kind="ExternalOutput").ap()

    with tile.TileContext(nc) as tc, ExitStack() as ctx:
        persist = ctx.enter_context(tc.tile_pool(name="persist", bufs=1))
        h_sb = persist.tile([128, N], BF16, tag="h")  # h[j,f] per j-tile
        rT_sb = persist.tile([128, N], BF16, tag="rT")  # residT[f,i]
        ciB = persist.tile([128, N], BF16, tag="ciB")  # ci bcast along partitions
        cjT = persist.tile([128, 2 * NJT], F32, tag="cjT")  # cj at even cols
        ones4_sb = persist.tile([128, 16], BF16, tag="ones4")
        bc4_sb = persist.tile([4, 512], BF16, tag="bc4")
        eye_sb = persist.tile([128, 128], BF16, tag="eye")
        nc.gpsimd.dma_start(ones4_sb[:], ones4_d)
        nc.gpsimd.dma_start(bc4_sb[:], bc4_d)
        nc.gpsimd.dma_start(eye_sb[:], eye_d)

        # Phase-2 pools opened first so their SBUF is disjoint from phase-1
        # scoped buffers.
        mpool = ctx.enter_context(tc.tile_pool(name="mpool", bufs=6))
        wpool = ctx.enter_context(tc.tile_pool(name="wpool", bufs=4))
        zpool = ctx.enter_context(tc.tile_pool(name="zpool", bufs=5))
        epool = ctx.enter_context(tc.tile_pool(name="epool", bufs=5))
        fin = ctx.enter_context(tc.tile_pool(name="fin", bufs=2))

        ph1 = ctx.enter_context(tc.tile_pool(name="ph1", bufs=1))
        xpool = ctx.enter_context(tc.tile_pool(name="xpool", bufs=8))
        hTp = ctx.enter_context(tc.tile_pool(name="hTp", bufs=2))

        # Sync queue: x tiles (PE-critical) then the mask stream; all small
        # constants ride the gpsimd queue.
        W_sb = ph1.tile([128, NMC * HF], BF16, tag="W")
        Wr_sb = ph1.tile([128, NMC * HF], BF16, tag="Wr")
        Wwi_sb = ph1.tile([128, NMC], BF16, tag="Wwi")
        xt0 = xpool.tile([128, HALF], BF16, tag="xt")
        nc.sync.dma_start(xt0[:, 0:1024], xT_d[0:128, 0:1024])
        nc.scalar.dma_start(xt0[:, 1024:HALF], xT_d[0:128, 1024:HALF])
        for mc in range(NMC):
            nc.gpsimd.dma_start(
                W_sb[:, mc * HF : (mc + 1) * HF], W_d[mc * 128 : (mc + 1) * 128, :]
            )
            nc.gpsimd.dma_start(
                Wwi_sb[:, mc : mc + 1], Wwi_d[mc * 128 : (mc + 1) * 128, :]
            )
        for mc in range(NMC):
            nc.gpsimd.dma_start(
                Wr_sb[:, mc * HF : (mc + 1) * HF],
                Wr_d[mc * 128 : (mc + 1) * 128, :],
            )
        wi_sb = ph1.tile([128, 1], BF16, tag="wi")
        nc.gpsimd.dma_start(wi_sb[:], wi_d)
        wj2_sb = ph1.tile([128, 2], BF16, tag="wj2")
        nc.gpsimd.dma_start(wj2_sb[:], wj2_d)

        # ---------- Phase 1: hT, ci, cj, h (residT moves to phase 2) -------
        xts_all = []
        for hf in range(2):
            o = hf * HALF
            hT_sb = hTp.tile([128, HALF], BF16, tag="hT")
            xts = []
            # Loop 1: hT. For half 0 only, ci rides in-stream (via the
            # host-precomputed W@w_i) so ciB is ready early and phase-2
            # elementwise starts while phase 1 still owns the PE.
            with ExitStack() as ps1:
                psA = ps1.enter_context(
                    tc.tile_pool(name=f"psA{hf}", bufs=1, space="PSUM")
                )
                ps_hT = psA.tile([128, HALF], F32, tag="ps_hT")
                ps_ci = (
                    psA.tile([1, HALF], F32, tag="ps_ci", name="ps_ci")
                    if hf == 0
                    else None
                )
                for mc in range(NMC):
                    if hf == 0 and mc == 0:
                        xt = xt0
                    else:
                        xt = xpool.tile([128, HALF], BF16, tag="xt")
                        nc.sync.dma_start(
                            xt[:], xT_d[mc * 128 : (mc + 1) * 128, o : o + HALF]
                        )
                    xts.append(xt)
                    for ck in range(HALF // 512):
                        nc.tensor.matmul(
                            ps_hT[:, ck * 512 : (ck + 1) * 512],
                            W_sb[:, mc * HF : (mc + 1) * HF],
                            xt[:, ck * 512 : (ck + 1) * 512],
                            start=(mc == 0),
                            stop=(mc == NMC - 1),
                        )
                    if hf == 0:
                        for ck in range(HALF // 512):
                            nc.tensor.matmul(
                                ps_ci[0:1, ck * 512 : (ck + 1) * 512],
                                Wwi_sb[:, mc : mc + 1],
                                xt[:, ck * 512 : (ck + 1) * 512],
                                start=(mc == 0),
                                stop=(mc == NMC - 1),
                            )
                # Evacuate split across engines so the PSUM frees fast.
                nc.scalar.copy(hT_sb[:, 0:1024], ps_hT[:, 0:1024])
                nc.vector.tensor_copy(hT_sb[:, 1024:HALF], ps_hT[:, 1024:HALF])
                if hf == 0:
                    ci_row = ph1.tile([1, HALF], BF16, tag="ci_row0")
                    nc.vector.tensor_copy(ci_row[:], ps_ci[:])
                    for c in range(4):
                        nc.gpsimd.partition_broadcast(
                            ciB[:, o + c * 512 : o + (c + 1) * 512],
                            ci_row[0:1, c * 512 : (c + 1) * 512],
                        )
            xts_all.append(xts)

            # Loop 2: cj, h, and (half 1) ci.
            with ExitStack() as ps2:
                psB = ps2.enter_context(
                    tc.tile_pool(name=f"psB{hf}", bufs=1, space="PSUM")
                )
                ps_cj = psB.tile([128, NJT], F32, tag="ps_cj")
                for k in range(NJT // 2):
                    nc.tensor.matmul(
                        ps_cj[:, 2 * k : 2 * k + 2],
                        hT_sb[:, k * 128 : (k + 1) * 128],
                        wj2_sb[:],
                        start=(k == 0),
                        stop=(k == NJT // 2 - 1),
                    )
                nc.vector.tensor_copy(cjT[:, hf * NJT : (hf + 1) * NJT], ps_cj[:])

                ps_h = psB.tile([128, HALF], BF16, tag="ps_h")
                for k in range(HALF // 128):
                    nc.tensor.transpose(
                        ps_h[:, k * 128 : (k + 1) * 128],
                        hT_sb[:, k * 128 : (k + 1) * 128],
                        eye_sb[:],
                    )
                nc.vector.tensor_copy(h_sb[:, o : o + HALF], ps_h[:])

                if hf == 1:
                    ps_ci1 = psB.tile([1, HALF], F32, tag="ps_ci1")
                    for c in range(4):
                        nc.tensor.matmul(
                            ps_ci1[0:1, c * 512 : (c + 1) * 512],
                            wi_sb[:],
                            hT_sb[:, c * 512 : (c + 1) * 512],
                            start=True,
                            stop=True,
                        )
                    ci_row1 = ph1.tile([1, HALF], BF16, tag="ci_row1")
                    nc.vector.tensor_copy(ci_row1[:], ps_ci1[:])
                    for c in range(4):
                        nc.gpsimd.partition_broadcast(
                            ciB[:, o + c * 512 : o + (c + 1) * 512],
                            ci_row1[0:1, c * 512 : (c + 1) * 512],
                        )

        # ---------- Phase 2: attention (+ interleaved residT chunks) -------
        rtp = ctx.enter_context(tc.tile_pool(name="rtp", bufs=2, space="PSUM"))
        rbp = ctx.enter_context(tc.tile_pool(name="rbp", bufs=1, space="PSUM"))
        for half in range(2):
            i0 = half * HALF
            with ExitStack() as pmm_ctx:
                pmm = pmm_ctx.enter_context(
                    tc.tile_pool(name=f"pmm{half}", bufs=1, space="PSUM")
                )
                yT_ps = [
                    pmm.tile([128, 512], F32, tag=f"yT{c}", name=f"yT_ps{c}")
                    for c in range(4)
                ]
                rs4_ps = pmm.tile([4, 512], F32, tag="rs4")

                for jt in range(NJT):
                    m_t = mpool.tile([128, HALF], BF16, tag="m")
                    nc.sync.dma_start(
                        m_t[:], mask_d[jt * 128 : (jt + 1) * 128, i0 : i0 + HALF]
                    )
                    w_t = wpool.tile([128, HALF], BF16, tag="w")
                    nc.vector.tensor_tensor(
                        w_t[:], m_t[:], ciB[:, i0 : i0 + HALF], op=OP.add
                    )
                    z_t = zpool.tile([128, HALF], BF16, tag="z")
                    nc.scalar.activation(
                        z_t[:],
                        w_t[:],
                        AF.Prelu,
                        bias=cjT[:, 2 * jt : 2 * jt + 1],
                        alpha=SLOPE,
                    )
                    e_t = epool.tile([128, HALF], I16, tag="e")
                    nc.vector.tensor_scalar(
                        e_t[:], z_t[:], A_EXP, B_EXP, op0=OP.mult, op1=OP.add
                    )
                    e_bf = e_t[:].bitcast(BF16)
                    hr = h_sb[:, jt * 128 : (jt + 1) * 128]
                    # rowsum first (tail-critical), one PSUM bank via the
                    # block-one-hot ones4 stationaries (chunk c -> row c).
                    for c in range(HALF // 512):
                        nc.tensor.matmul(
                            rs4_ps[0:4, :],
                            ones4_sb[:, c * 4 : (c + 1) * 4],
                            e_bf[:, c * 512 : (c + 1) * 512],
                            start=(jt == 0 and c == 0),
                            stop=(jt == NJT - 1 and c == 3),
                        )
                    for c in range(HALF // 512):
                        nc.tensor.matmul(
                            yT_ps[c][:],
                            hr,
                            e_bf[:, c * 512 : (c + 1) * 512],
                            start=(jt == 0),
                            stop=(jt == NJT - 1),
                        )
                    # residT chunk k of this half, slotted into PE slack
                    # (PSUM banks 6-7 cycle through the rtp pool).
                    if jt % 8 == 4:
                        k = jt // 8
                        co = i0 + k * 512
                        rt_ps = rtp.tile([128, 512], F32, tag="rt")
                        for mc in range(NMC):
                            nc.tensor.matmul(
                                rt_ps[:],
                                Wr_sb[:, mc * HF : (mc + 1) * HF],
                                xts_all[half][mc][:, k * 512 : (k + 1) * 512],
                                start=(mc == 0),
                                stop=(mc == NMC - 1),
                            )
                        nc.vector.tensor_copy(rT_sb[:, co : co + 512], rt_ps[:])

                # Finale: chunked yT evac; 1/rs via approx recip; broadcast
                # by PE outer-product (bc4 row-one-hot stationary); normalize
                # and DMA per 512-col chunk.
                yT_sb = fin.tile([128, HALF], BF16, tag="yT_sb")
                for c in range(4):
                    nc.scalar.copy(yT_sb[:, c * 512 : (c + 1) * 512], yT_ps[c][:])
                recip4 = fin.tile([4, 512], F32, tag="recip4")
                nc.vector.reciprocal_approx_fast(recip4[:], rs4_ps[0:4, :])
                recip4b = fin.tile([4, 512], BF16, tag="recip4b")
                nc.vector.tensor_copy(recip4b[:], recip4[:])
                ytn = fin.tile([128, HALF], BF16, tag="ytn")
                for c in range(4):
                    sl = slice(c * 512, (c + 1) * 512)
                    rb_ps = rbp.tile([128, 512], F32, tag="rb")
                    nc.tensor.matmul(
                        rb_ps[:],
                        bc4_sb[0:4, c * 128 : (c + 1) * 128],
                        recip4b[0:4, :],
                        start=True,
                        stop=True,
                    )
                    nc.vector.tensor_tensor(
                        ytn[:, sl], yT_sb[:, sl], rb_ps[:], op=OP.mult
                    )
                    nc.vector.tensor_tensor(
                        ytn[:, sl],
                        ytn[:, sl],
                        rT_sb[:, i0 + c * 512 : i0 + (c + 1) * 512],
                        op=OP.add,
                    )
                    nc.scalar.dma_start(
                        y_d[:, i0 + c * 512 : i0 + (c + 1) * 512], ytn[:, sl]
                    )

    nc.compile()
    return nc


def _get_program():
    global _prog
    if _prog is None:
        _prog = build_program()
    return _prog


def _prepare_in_maps(x, graph, W, w_i, w_j, W_r):
    bf = ml_dtypes.bfloat16
    xT = np.ascontiguousarray(x.T).astype(bf)
    mask = np.where(graph > 0, np.float32(0.0), np.float32(MASK_NEG)).astype(bf)
    eye = np.eye(128, dtype=np.float32).astype(bf)
    ones4 = np.zeros((128, 16), dtype=np.float32)
    for c in range(4):
        ones4[:, 4 * c + c] = 1.0
    ones4 = ones4.astype(bf)
    bc4 = np.zeros((4, 512), dtype=np.float32)
    for c in range(4):
        bc4[c, c * 128 : (c + 1) * 128] = 1.0
    bc4 = bc4.astype(bf)
    in_maps = []
    for c in range(HEADS):
        wj2 = np.zeros((HF, 2), dtype=np.float32)
        wj2[:, 0] = np.asarray(w_j[c], dtype=np.float32).reshape(HF)
        in_maps.append(
            {
                "xT": xT,
                "mask": mask,
                "W": np.ascontiguousarray(W[c]).astype(bf),
                "Wr": np.ascontiguousarray(W_r[:, c * HF : (c + 1) * HF]).astype(bf),
                "Wwi": (
                    np.asarray(W[c], dtype=np.float32)
                    @ np.asarray(w_i[c], dtype=np.float32)
                ).astype(bf),
                "wi": np.asarray(w_i[c], dtype=np.float32).astype(bf),
                "wj2": wj2.astype(bf),
                "ones4": ones4,
                "bc4": bc4,
                "eye": eye,
            }
        )
    return in_maps


def run(inputs, trace=False, **kwargs):
    """Run the SPMD kernel; returns (y_full, BassKernelResults)."""
    x = np.asarray(inputs["x"], dtype=np.float32)
    graph = np.asarray(inputs["graph"])
    W = np.asarray(inputs["W"], dtype=np.float32)
    w_i = np.asarray(inputs["w_i"], dtype=np.float32)
    w_j = np.asarray(inputs["w_j"], dtype=np.float32)
    W_r = np.asarray(inputs["W_r"], dtype=np.float32)
    bias = np.asarray(inputs["bias"], dtype=np.float32)

    nc = _get_program()
    in_maps = _prepare_in_maps(x, graph, W, w_i, w_j, W_r)
    br = run_bass_kernel_spmd(
        nc, in_maps, core_ids=list(range(HEADS)), trace=trace, **kwargs
    )
    y = np.concatenate(
        [br.results[c]["y"].astype(np.float32).T for c in range(HEADS)], axis=1
    )
    y = y + bias[None, :]
    return y.astype(np.float32), br


def kernel(**inputs):
    y, _ = run(inputs)
    return y
